# revision 26
# baseline (speedup 1.0000x reference)
"""GroupedQueryAttention Trainium2 kernel (8 NeuronCores, SPMD).

Sharding: 16 (batch, q-head) pairs over 8 cores -> core c handles batch c//4,
kv-head j=c%4, q-heads {2j, 2j+1}. Each core computes its heads' causal flash
attention plus its partial output projection; host sums the 4 partials per
batch.

Device-side layout is fully "transposed" (head_dim on partitions, sequence on
free dim) so no on-chip transposes are needed anywhere:
  scores^T[k, q] = kT_chunk.T @ qT   (4x row-tiled matmuls, K=32)
  P^T = exp(scores^T * 1/sqrt(hd))   (single fused ACT op per 512-k group)
  out^T[hd, q]  = v_aug.T @ P^T      (v_aug has a leading ones column ->
                                      row 0 of the accumulator is the softmax
                                      denominator, for free)
RoPE is applied with zero shuffles by projecting hidden states twice: once
with W and once with (perm+sign) W, then q' = qT*cos + qrotT*sin.

All matmul inputs are bf16 (1 PE cycle/row vs 4 for fp32); accumulation
stays fp32 in PSUM.

The per-block softmax normalization + output projection is software-pipelined
one block behind the attention loop: the slow DVE reciprocal (~3.3us) runs
while the next block's score/PV matmuls keep the Tensor engine busy, so the
HAM clock gate stays at 2.4GHz instead of resetting to 1.2GHz at every block
boundary.
"""

import json
import sys

import numpy as np

for _p in ("/opt/trn_rl_repo",):
    if _p not in sys.path:
        try:
            import concourse.bass  # noqa: F401
        except Exception:
            sys.path.insert(0, _p)
    break

import concourse.bass as bass
import concourse.tile as tile
from concourse import mybir
from concourse.bass_utils import run_bass_kernel_spmd

F32 = mybir.dt.float32
BF16 = mybir.dt.bfloat16
NP_BF16 = mybir.dt.np(BF16)

B, S, H = 2, 4096, 256
NH, NKV, HD = 8, 4, 32
QB = 512                   # q block width
NQB = S // QB              # 8
KC = 128                   # k chunk
SCALE = 1.0 / np.sqrt(HD)
ROPE_BASE = 10000.0


# ---------------------------------------------------------------- wait fixup
def _fix_waits_json(bir_bytes: bytes) -> bytes:
    """walrus (gen3) allows only one sync-wait per instruction struct; hoist
    extra waits onto inserted same-engine NoOps."""
    m = json.loads(bir_bytes)
    counter = 0
    for f in m.get("functions", []):
        for blk in f.get("blocks", []):
            out = []
            for inst in blk.get("instructions", []):
                si = inst.get("sync_info") or {}
                waits = si.get("on_wait") or []
                keep = 0 if inst.get("opcode") == "Matmult" else 1
                if len(waits) > keep:
                    for wsub in waits[keep:]:
                        counter += 1
                        out.append({
                            "debug": inst.get("debug", 0),
                            "engine": inst["engine"],
                            "ins": [],
                            "outs": [],
                            "name": f"waitfix-{counter}",
                            "opcode": "NoOp",
                            "sync_info": {"on_update": [], "on_wait": [wsub]},
                        })
                    si["on_wait"] = waits[:keep]
                out.append(inst)
            blk["instructions"] = out
    return json.dumps(m).encode()


def _install_waitfix(nc):
    orig = nc.to_json_bytes

    def patched(*a, **k):
        return _fix_waits_json(orig(*a, **k))

    nc.to_json_bytes = patched


# ---------------------------------------------------------------- device code
def _build_module():
    nc = bass.Bass()

    hsT = nc.declare_dram_parameter("hsT", [H, S], BF16, isOutput=False)
    wqkT = nc.declare_dram_parameter("wqkT", [H, 96], BF16, isOutput=False)
    wqkrotT = nc.declare_dram_parameter("wqkrotT", [H, 96], BF16, isOutput=False)
    wvT = nc.declare_dram_parameter("wvT", [H, HD], BF16, isOutput=False)
    gt0 = nc.declare_dram_parameter("gt0", [HD + 1, H], BF16, isOutput=False)
    gt1 = nc.declare_dram_parameter("gt1", [HD + 1, H], BF16, isOutput=False)
    cosT = nc.declare_dram_parameter("cosT", [96, S], BF16, isOutput=False)
    sinT = nc.declare_dram_parameter("sinT", [96, S], BF16, isOutput=False)
    tri = nc.declare_dram_parameter("tri", [KC, KC], BF16, isOutput=False)
    out_part = nc.declare_dram_parameter("out_part", [H, S], BF16, isOutput=True)

    with tile.TileContext(nc) as tc:
        with (
            tc.tile_pool(name="const", bufs=1) as const,
            tc.tile_pool(name="qtp", bufs=4) as qtp,
            tc.tile_pool(name="qkp", bufs=4) as qkp,
            tc.tile_pool(name="ptp", bufs=4) as ptp,
            tc.tile_pool(name="smallp", bufs=6) as smallp,
            tc.tile_pool(name="up", bufs=4) as up,
            tc.tile_pool(name="ntp", bufs=4) as ntp,
            tc.tile_pool(name="outp", bufs=3) as outp,
            tc.tile_pool(name="ps_sc", bufs=2, space="PSUM") as ps_sc,
            tc.tile_pool(name="ps_pv", bufs=2, space="PSUM") as ps_pv,
            tc.tile_pool(name="ps_mm", bufs=2, space="PSUM") as ps_mm,
        ):
            # ---- persistent tiles
            hsT_sb = const.tile([128, 2, S], BF16)
            kT_rep = const.tile([128, S], BF16)
            v_all = const.tile([128, S // KC, HD + 1], BF16)
            cos_sb = const.tile([96, S], BF16)
            sin_sb = const.tile([96, S], BF16)
            tri_sb = const.tile([KC, KC], BF16)
            wqkT_sb = const.tile([128, 2, 96], BF16)
            wqkrotT_sb = const.tile([128, 2, 96], BF16)
            wvT_sb = const.tile([128, 2, HD], BF16)
            gt0_sb = const.tile([HD + 1, 2, 128], BF16)
            gt1_sb = const.tile([HD + 1, 2, 128], BF16)
            ones1 = const.tile([1, HD + 1], BF16)

            # ---- PE warm-up: back-to-back matmuls trip the HAM clock gate
            # from 1.2GHz to 2.4GHz while the prologue DMAs fly.
            wtmp = const.tile([128, QB], BF16)
            nc.vector.memset(wtmp[:], 0.0)
            for w in range(24):
                pwarm = ps_mm.tile([128, QB], F32, tag="mm", name="warm")
                nc.tensor.matmul(pwarm[:], wtmp[:, 0:128], wtmp[:],
                                 start=True, stop=True)

            # ---- prologue DMAs, ordered by when block 0 needs them:
            # hidden states + Wqk first, then block-0 rope tables, the rest.
            for c in range(2):
                nc.sync.dma_start(out=hsT_sb[:, c, :],
                                  in_=hsT[128 * c:128 * (c + 1), :])
            for c in range(2):
                nc.sync.dma_start(out=wqkT_sb[:, c, :], in_=wqkT[128 * c:128 * (c + 1), :])
                nc.sync.dma_start(out=wqkrotT_sb[:, c, :], in_=wqkrotT[128 * c:128 * (c + 1), :])
            nc.sync.dma_start(out=cos_sb[:, 0:1024], in_=cosT[:, 0:1024])
            nc.sync.dma_start(out=sin_sb[:, 0:1024], in_=sinT[:, 0:1024])
            for c in range(2):
                nc.sync.dma_start(out=wvT_sb[:, c, :], in_=wvT[128 * c:128 * (c + 1), :])
                nc.sync.dma_start(out=gt0_sb[:, c, :], in_=gt0[:, 128 * c:128 * (c + 1)])
                nc.sync.dma_start(out=gt1_sb[:, c, :], in_=gt1[:, 128 * c:128 * (c + 1)])
            nc.sync.dma_start(out=tri_sb[:], in_=tri[:])
            nc.vector.memset(ones1[:], 1.0)
            nc.vector.memset(v_all[:, :, 0:1], 1.0)
            for ch4 in range(1, 4):
                csl = slice(1024 * ch4, 1024 * (ch4 + 1))
                nc.sync.dma_start(out=cos_sb[:, csl], in_=cosT[:, csl])
                nc.sync.dma_start(out=sin_sb[:, csl], in_=sinT[:, csl])

            # deferred-normalization state of the previous block
            pipe = {}

            def emit_norm(qbp):
                """normalize + output-project block qbp (reciprocals already
                issued; runs while the current block's attention keeps PE hot)."""
                st = pipe.pop(qbp)
                slp = slice(QB * qbp, QB * (qbp + 1))
                nT = [None, None]
                for h in range(2):
                    bc_ps = ps_mm.tile([HD + 1, QB], F32, tag="mm", name="bc")
                    nc.tensor.matmul(bc_ps[:], ones1[:], st["rc"][h][:],
                                     start=True, stop=True)
                    bcs = smallp.tile([HD + 1, QB], BF16, tag="bcs", name="bcs")
                    with nc.allow_low_precision(reason="softmax denom bcast"):
                        nc.vector.tensor_copy(bcs[:], bc_ps[:])
                    nT[h] = ntp.tile([HD + 1, QB], BF16, tag=f"nT{h}",
                                     name=f"nT{h}")
                    nc.vector.tensor_mul(nT[h][:], st["u"][h][:], bcs[:])
                for mchunk in range(2):
                    po = ps_mm.tile([128, QB], F32, tag="mm", name="outproj")
                    nc.tensor.matmul(po[:], gt0_sb[:, mchunk, :], nT[0][:],
                                     start=True, stop=False)
                    nc.tensor.matmul(po[:], gt1_sb[:, mchunk, :], nT[1][:],
                                     start=False, stop=True)
                    po_sb = outp.tile([128, QB], BF16)
                    nc.vector.tensor_copy(po_sb[:], po[:])
                    nc.sync.dma_start(
                        out=out_part[128 * mchunk:128 * (mchunk + 1), slp],
                        in_=po_sb[:])

            qt_state = {}

            def emit_block_prologue(qb):
                """q/k/v projections + RoPE + band replication for block qb.
                Called from inside block qb-1's attention so the boundary has
                no PE idle (the HAM clock gate cools after ~3.4us idle)."""
                sl = slice(QB * qb, QB * (qb + 1))
                p_qk = ps_mm.tile([96, QB], F32, tag="mm", name="p_qk")
                p_qkr = ps_mm.tile([96, QB], F32, tag="mm", name="p_qkr")
                for c in range(2):
                    nc.tensor.matmul(p_qk[:], wqkT_sb[:, c, :], hsT_sb[:, c, sl],
                                     start=(c == 0), stop=(c == 1))
                for c in range(2):
                    nc.tensor.matmul(p_qkr[:], wqkrotT_sb[:, c, :], hsT_sb[:, c, sl],
                                     start=(c == 0), stop=(c == 1))
                qkT = qkp.tile([96, QB], BF16, tag="qkT")
                rtmp = qkp.tile([96, QB], BF16, tag="rtmp")
                nc.vector.tensor_mul(qkT[:], p_qk[:], cos_sb[:, sl])
                nc.vector.tensor_mul(rtmp[:], p_qkr[:], sin_sb[:, sl])
                nc.vector.tensor_add(qkT[:], qkT[:], rtmp[:])

                # replicate qT (per head) and kT across the 4 row bands
                qt_h = [qtp.tile([128, QB], BF16, tag=f"qt{h}", name=f"qt{h}")
                        for h in range(2)]
                for t in range(4):
                    dst = slice(32 * t, 32 * (t + 1))
                    nc.sync.dma_start(out=qt_h[0][dst, :], in_=qkT[0:32, :])
                    nc.sync.dma_start(out=qt_h[1][dst, :], in_=qkT[32:64, :])
                    nc.sync.dma_start(out=kT_rep[dst, sl], in_=qkT[64:96, :])

                # v projection for this block's 4 s-chunks
                for s4 in range(4):
                    sblk = 4 * qb + s4
                    pv_ps = ps_mm.tile([128, HD], F32, tag="mm", name="vproj")
                    for c in range(2):
                        nc.tensor.matmul(
                            pv_ps[:], hsT_sb[:, c, KC * sblk:KC * (sblk + 1)],
                            wvT_sb[:, c, :], start=(c == 0), stop=(c == 1))
                    nc.vector.tensor_copy(v_all[:, sblk, 1:], pv_ps[:])
                qt_state[qb] = qt_h

            emit_block_prologue(0)

            for qb in range(NQB):
                q0 = QB * qb
                sl = slice(q0, q0 + QB)
                qt_h = qt_state.pop(qb)

                # ---- previous block: reciprocals (slow DVE) issued at block
                # head; consumed by emit_norm at step dp
                if qb > 0:
                    st = pipe[qb - 1]
                    for h in range(2):
                        rc = smallp.tile([1, QB], BF16, tag="rc", name="rc")
                        with nc.allow_low_precision(reason="bf16 softmax denom"):
                            nc.vector.reciprocal(out=rc[:], in_=st["dn"][h][:])
                        st["rc"].append(rc)

                # ---- attention per head
                ngroups = 2 * (qb + 1)
                # defer the previous block's norm far enough into this block's
                # attention that the reciprocals have finished on DVE
                dp = min(ngroups - 1, 9) if qb > 0 else -1
                pvaccs = [ps_pv.tile([HD + 1, QB], F32, tag="pvacc",
                                     name=f"pvacc{h}") for h in range(2)]

                def emit_pv(st_):
                    g_, h_, pt_ = st_
                    for d in range(2):
                        ch = 2 * g_ + d
                        dg = ch - 4 * qb
                        diag_ = (g_ >= 2 * qb)
                        cols = KC * dg if diag_ else 0
                        nc.tensor.matmul(
                            pvaccs[h_][:, cols:],
                            v_all[:, ch, :],
                            pt_[:, QB * d + cols:QB * (d + 1)],
                            start=(g_ == 0 and d == 0),
                            stop=(g_ == 2 * qb + 1 and d == 1),
                            skip_group_check=True)

                # PV pairs run two steps behind their scores so the PE never
                # waits on the exp: while act(i) runs, the PE does pv(i-2),
                # sc(i+1), pv(i-1) -- it reaches pv(i) well after act(i) ends.
                defer = []
                for g in range(ngroups):
                    for h in range(2):
                        sc = ps_sc.tile([128, 2 * QB], F32, tag="sc", name="sc")
                        # causal: chunk dg of the diagonal group only needs
                        # q >= KC*dg; trim matmul + act to that region
                        qlo = [0, 0]
                        if g == 2 * qb:
                            qlo = [0, KC]
                        elif g == 2 * qb + 1:
                            qlo = [2 * KC, 3 * KC]
                        for d in range(2):
                            ch = 2 * g + d
                            t = ch % 4
                            nc.tensor.matmul(
                                sc[:, QB * d + qlo[d]:QB * (d + 1)],
                                kT_rep[32 * t:32 * (t + 1), KC * ch:KC * (ch + 1)],
                                qt_h[h][32 * t:32 * (t + 1), qlo[d]:],
                                start=True, stop=True, tile_position=(32 * t, 0))
                        pt = ptp.tile([128, 2 * QB], BF16, tag="pt", name="pt")
                        nc.scalar.activation(out=pt[:, qlo[0]:], in_=sc[:, qlo[0]:],
                                             func=mybir.ActivationFunctionType.Exp,
                                             scale=float(SCALE))
                        if g >= 2 * qb:
                            for d in range(2):
                                dg = 2 * g + d - 4 * qb
                                w0 = QB * d + KC * dg
                                nc.vector.tensor_mul(
                                    pt[:, w0:w0 + KC], pt[:, w0:w0 + KC], tri_sb[:])
                        defer.append((g, h, pt))
                        if len(defer) > 2:
                            emit_pv(defer.pop(0))
                    if g == dp:
                        emit_norm(qb - 1)
                    if g == ngroups - 2 and qb + 1 < NQB:
                        emit_block_prologue(qb + 1)
                for st_ in defer:
                    emit_pv(st_)

                # ---- epilogue: cheap DVE reads free the PSUM accumulators
                # fast; the reciprocal itself is deferred into the next block
                st = {"u": [], "dn": [], "rc": []}
                for h in range(2):
                    u = up.tile([HD + 1, QB], BF16, tag=f"u{h}", name=f"u{h}")
                    with nc.allow_low_precision(reason="unnorm attn bf16"):
                        nc.vector.tensor_copy(u[:], pvaccs[h][:])
                    dn = smallp.tile([1, QB], F32, tag="dn", name="dn")
                    nc.vector.tensor_copy(dn[:], pvaccs[h][0:1, :])
                    st["u"].append(u)
                    st["dn"].append(dn)
                pipe[qb] = st

            # ---- final block: reciprocal + norm tail
            st = pipe[NQB - 1]
            for h in range(2):
                rc = smallp.tile([1, QB], BF16, tag="rc", name="rc")
                with nc.allow_low_precision(reason="bf16 softmax denom"):
                    nc.vector.reciprocal(out=rc[:], in_=st["dn"][h][:])
                st["rc"].append(rc)
            emit_norm(NQB - 1)

    _install_waitfix(nc)
    return nc


_NC_CACHE = {}


def _get_nc():
    if "nc" not in _NC_CACHE:
        _NC_CACHE["nc"] = _build_module()
    return _NC_CACHE["nc"]


# ---------------------------------------------------------------- host side
def _rope_tables():
    inv = 1.0 / (ROPE_BASE ** (np.arange(0, HD, 2, dtype=np.float64) / HD))
    t = np.arange(S, dtype=np.float64)
    freqs = np.outer(t, inv)                     # [S, 16]
    emb = np.concatenate([freqs, freqs], axis=1)  # [S, 32]
    cosT = np.cos(emb).T.astype(np.float32)      # [32, S]
    sinT = np.sin(emb).T.astype(np.float32)
    return np.tile(cosT, (3, 1)), np.tile(sinT, (3, 1))   # [96, S]


def _rot_rows(w):
    # rows of (rotate_half o) projection: row d<16 -> -w[d+16]; d>=16 -> w[d-16]
    return np.concatenate([-w[16:32], w[0:16]], axis=0)


def kernel(hidden_states, Wq, Wk, Wv, Wo):
    hidden_states = np.asarray(hidden_states, dtype=np.float32)
    Wq = np.asarray(Wq, dtype=np.float32)
    Wk = np.asarray(Wk, dtype=np.float32)
    Wv = np.asarray(Wv, dtype=np.float32)
    Wo = np.asarray(Wo, dtype=np.float32)

    cosT, sinT = _rope_tables()
    tri = (np.arange(KC)[:, None] <= np.arange(KC)[None, :]).astype(NP_BF16)

    hsT_b = [np.ascontiguousarray(hidden_states[b].T).astype(NP_BF16)
             for b in range(B)]

    in_maps = []
    for core in range(8):
        b, j = core // 4, core % 4
        Wq_h = Wq[64 * j:64 * j + 64]            # [64, 256]
        Wk_j = Wk[32 * j:32 * j + 32]            # [32, 256]
        Wqk = np.concatenate([Wq_h, Wk_j], axis=0)           # [96, 256]
        Wqkrot = np.concatenate(
            [_rot_rows(Wq_h[0:32]), _rot_rows(Wq_h[32:64]), _rot_rows(Wk_j)],
            axis=0)
        G = Wo[:, 64 * j:64 * j + 64]            # [256, 64]
        gt0 = np.zeros((HD + 1, H), np.float32)
        gt0[1:] = G[:, 0:32].T
        gt1 = np.zeros((HD + 1, H), np.float32)
        gt1[1:] = G[:, 32:64].T
        in_maps.append({
            "hsT": hsT_b[b],
            "wqkT": np.ascontiguousarray(Wqk.T).astype(NP_BF16),
            "wqkrotT": np.ascontiguousarray(Wqkrot.T).astype(NP_BF16),
            "wvT": np.ascontiguousarray(Wv[32 * j:32 * j + 32].T).astype(NP_BF16),
            "gt0": gt0.astype(NP_BF16),
            "gt1": gt1.astype(NP_BF16),
            "cosT": cosT.astype(NP_BF16),
            "sinT": sinT.astype(NP_BF16),
            "tri": tri,
        })

    nc = _get_nc()
    res = run_bass_kernel_spmd(nc, in_maps, list(range(8)), trace=False)

    out = np.empty((B, S, H), np.float32)
    for b in range(B):
        acc = np.zeros((H, S), np.float32)
        for j in range(4):
            acc += np.asarray(res.results[4 * b + j]["out_part"],
                              dtype=np.float32)
        out[b] = acc.T
    return out


# revision 27
# speedup vs baseline: 1.1101x; 1.1101x over previous
"""GroupedQueryAttention Trainium2 kernel (8 NeuronCores, SPMD).

Sharding: 16 (batch, q-head) pairs over 8 cores -> core c handles batch c//4,
kv-head j=c%4, q-heads {2j, 2j+1}. Each core computes its heads' causal flash
attention plus its partial output projection; host sums the 4 partials per
batch.

Device-side layout is fully "transposed" (head_dim on partitions, sequence on
free dim) so no on-chip transposes are needed anywhere:
  scores^T[k, q] = kT_chunk.T @ qT   (4x row-tiled matmuls, K=32)
  P^T = exp(scores^T * 1/sqrt(hd))   (single fused ACT op per 512-k group)
  out^T[hd, q]  = v_aug.T @ P^T      (v_aug has a leading ones column ->
                                      row 0 of the accumulator is the softmax
                                      denominator, for free)
RoPE is applied with zero shuffles by projecting hidden states twice: once
with W and once with (perm+sign) W, then q' = qT*cos + qrotT*sin.

All matmul inputs are bf16 (1 PE cycle/row vs 4 for fp32); accumulation
stays fp32 in PSUM.

The per-block softmax normalization + output projection is software-pipelined
one block behind the attention loop: the slow DVE reciprocal (~3.3us) runs
while the next block's score/PV matmuls keep the Tensor engine busy, so the
HAM clock gate stays at 2.4GHz instead of resetting to 1.2GHz at every block
boundary.
"""

import json
import sys

import numpy as np

for _p in ("/opt/trn_rl_repo",):
    if _p not in sys.path:
        try:
            import concourse.bass  # noqa: F401
        except Exception:
            sys.path.insert(0, _p)
    break

import concourse.bass as bass
import concourse.tile as tile
from concourse import mybir
from concourse.bass_utils import run_bass_kernel_spmd

F32 = mybir.dt.float32
BF16 = mybir.dt.bfloat16
NP_BF16 = mybir.dt.np(BF16)

B, S, H = 2, 4096, 256
NH, NKV, HD = 8, 4, 32
QB = 512                   # q block width
NQB = S // QB              # 8
KC = 128                   # k chunk
SCALE = 1.0 / np.sqrt(HD)
ROPE_BASE = 10000.0


# ---------------------------------------------------------------- wait fixup
def _fix_waits_json(bir_bytes: bytes) -> bytes:
    """walrus (gen3) allows only one sync-wait per instruction struct; hoist
    extra waits onto inserted same-engine NoOps."""
    m = json.loads(bir_bytes)
    counter = 0
    for f in m.get("functions", []):
        for blk in f.get("blocks", []):
            out = []
            for inst in blk.get("instructions", []):
                si = inst.get("sync_info") or {}
                waits = si.get("on_wait") or []
                keep = 0 if inst.get("opcode") == "Matmult" else 1
                if len(waits) > keep:
                    for wsub in waits[keep:]:
                        counter += 1
                        out.append({
                            "debug": inst.get("debug", 0),
                            "engine": inst["engine"],
                            "ins": [],
                            "outs": [],
                            "name": f"waitfix-{counter}",
                            "opcode": "NoOp",
                            "sync_info": {"on_update": [], "on_wait": [wsub]},
                        })
                    si["on_wait"] = waits[:keep]
                out.append(inst)
            blk["instructions"] = out
    return json.dumps(m).encode()


def _install_waitfix(nc):
    orig = nc.to_json_bytes

    def patched(*a, **k):
        return _fix_waits_json(orig(*a, **k))

    nc.to_json_bytes = patched


# ---------------------------------------------------------------- device code
def _build_module():
    nc = bass.Bass()

    hsT = nc.declare_dram_parameter("hsT", [H, S], BF16, isOutput=False)
    wqkT = nc.declare_dram_parameter("wqkT", [H, 96], BF16, isOutput=False)
    wqkrotT = nc.declare_dram_parameter("wqkrotT", [H, 96], BF16, isOutput=False)
    wvT = nc.declare_dram_parameter("wvT", [H, HD], BF16, isOutput=False)
    gt0 = nc.declare_dram_parameter("gt0", [HD + 1, H], BF16, isOutput=False)
    gt1 = nc.declare_dram_parameter("gt1", [HD + 1, H], BF16, isOutput=False)
    cosT = nc.declare_dram_parameter("cosT", [96, S], BF16, isOutput=False)
    sinT = nc.declare_dram_parameter("sinT", [96, S], BF16, isOutput=False)
    tri = nc.declare_dram_parameter("tri", [KC, KC], BF16, isOutput=False)
    out_part = nc.declare_dram_parameter("out_part", [H, S], BF16, isOutput=True)

    with tile.TileContext(nc) as tc:
        with (
            tc.tile_pool(name="const", bufs=1) as const,
            tc.tile_pool(name="qtp", bufs=4) as qtp,
            tc.tile_pool(name="qkp", bufs=4) as qkp,
            tc.tile_pool(name="ptp", bufs=4) as ptp,
            tc.tile_pool(name="smallp", bufs=6) as smallp,
            tc.tile_pool(name="up", bufs=4) as up,
            tc.tile_pool(name="ntp", bufs=4) as ntp,
            tc.tile_pool(name="outp", bufs=3) as outp,
            tc.tile_pool(name="ps_sc", bufs=2, space="PSUM") as ps_sc,
            tc.tile_pool(name="ps_pv", bufs=2, space="PSUM") as ps_pv,
            tc.tile_pool(name="ps_mm", bufs=2, space="PSUM") as ps_mm,
        ):
            # ---- persistent tiles
            hsT_sb = const.tile([128, 2, S], BF16)
            kT_rep = const.tile([128, S], BF16)
            v_all = const.tile([128, S // KC, HD + 1], BF16)
            cos_sb = const.tile([96, S], BF16)
            sin_sb = const.tile([96, S], BF16)
            tri_sb = const.tile([KC, KC], BF16)
            wqkT_sb = const.tile([128, 2, 96], BF16)
            wqkrotT_sb = const.tile([128, 2, 96], BF16)
            wvT_sb = const.tile([128, 2, HD], BF16)
            gt0_sb = const.tile([HD + 1, 2, 128], BF16)
            gt1_sb = const.tile([HD + 1, 2, 128], BF16)
            ones1 = const.tile([1, HD + 1], BF16)

            # ---- PE warm-up: back-to-back matmuls trip the HAM clock gate
            # from 1.2GHz to 2.4GHz while the prologue DMAs fly.
            wtmp = const.tile([128, QB], BF16)
            nc.vector.memset(wtmp[:], 0.0)
            for w in range(24):
                pwarm = ps_mm.tile([128, QB], F32, tag="mm", name="warm")
                nc.tensor.matmul(pwarm[:], wtmp[:, 0:128], wtmp[:],
                                 start=True, stop=True)

            # ---- prologue DMAs, ordered by when block 0 needs them:
            # hidden states + Wqk first, then block-0 rope tables, the rest.
            for c in range(2):
                nc.sync.dma_start(out=hsT_sb[:, c, :],
                                  in_=hsT[128 * c:128 * (c + 1), :])
            for c in range(2):
                nc.sync.dma_start(out=wqkT_sb[:, c, :], in_=wqkT[128 * c:128 * (c + 1), :])
                nc.sync.dma_start(out=wqkrotT_sb[:, c, :], in_=wqkrotT[128 * c:128 * (c + 1), :])
            nc.sync.dma_start(out=cos_sb[:, 0:1024], in_=cosT[:, 0:1024])
            nc.sync.dma_start(out=sin_sb[:, 0:1024], in_=sinT[:, 0:1024])
            for c in range(2):
                nc.sync.dma_start(out=wvT_sb[:, c, :], in_=wvT[128 * c:128 * (c + 1), :])
                nc.sync.dma_start(out=gt0_sb[:, c, :], in_=gt0[:, 128 * c:128 * (c + 1)])
                nc.sync.dma_start(out=gt1_sb[:, c, :], in_=gt1[:, 128 * c:128 * (c + 1)])
            nc.sync.dma_start(out=tri_sb[:], in_=tri[:])
            nc.vector.memset(ones1[:], 1.0)
            nc.vector.memset(v_all[:, :, 0:1], 1.0)
            for ch4 in range(1, 4):
                csl = slice(1024 * ch4, 1024 * (ch4 + 1))
                nc.sync.dma_start(out=cos_sb[:, csl], in_=cosT[:, csl])
                nc.sync.dma_start(out=sin_sb[:, csl], in_=sinT[:, csl])

            # deferred-normalization state of the previous block
            pipe = {}

            def emit_norm(qbp):
                """normalize + output-project block qbp (reciprocals already
                issued; runs while the current block's attention keeps PE hot)."""
                st = pipe.pop(qbp)
                slp = slice(QB * qbp, QB * (qbp + 1))
                nT = [None, None]
                for h in range(2):
                    bc_ps = ps_mm.tile([HD + 1, QB], F32, tag="mm", name="bc")
                    nc.tensor.matmul(bc_ps[:], ones1[:], st["rc"][h][:],
                                     start=True, stop=True)
                    bcs = smallp.tile([HD + 1, QB], BF16, tag="bcs", name="bcs")
                    with nc.allow_low_precision(reason="softmax denom bcast"):
                        nc.vector.tensor_copy(bcs[:], bc_ps[:])
                    nT[h] = ntp.tile([HD + 1, QB], BF16, tag=f"nT{h}",
                                     name=f"nT{h}")
                    nc.vector.tensor_mul(nT[h][:], st["u"][h][:], bcs[:])
                for mchunk in range(2):
                    po = ps_mm.tile([128, QB], F32, tag="mm", name="outproj")
                    nc.tensor.matmul(po[:], gt0_sb[:, mchunk, :], nT[0][:],
                                     start=True, stop=False)
                    nc.tensor.matmul(po[:], gt1_sb[:, mchunk, :], nT[1][:],
                                     start=False, stop=True)
                    po_sb = outp.tile([128, QB], BF16)
                    nc.vector.tensor_copy(po_sb[:], po[:])
                    nc.sync.dma_start(
                        out=out_part[128 * mchunk:128 * (mchunk + 1), slp],
                        in_=po_sb[:])

            qt_state = {}

            def emit_block_prologue(qb):
                """q/k/v projections + RoPE + band replication for block qb.
                Called from inside block qb-1's attention so the boundary has
                no PE idle (the HAM clock gate cools after ~3.4us idle)."""
                sl = slice(QB * qb, QB * (qb + 1))
                p_qk = ps_mm.tile([96, QB], F32, tag="mm", name="p_qk")
                p_qkr = ps_mm.tile([96, QB], F32, tag="mm", name="p_qkr")
                for c in range(2):
                    nc.tensor.matmul(p_qk[:], wqkT_sb[:, c, :], hsT_sb[:, c, sl],
                                     start=(c == 0), stop=(c == 1))
                for c in range(2):
                    nc.tensor.matmul(p_qkr[:], wqkrotT_sb[:, c, :], hsT_sb[:, c, sl],
                                     start=(c == 0), stop=(c == 1))
                qkT = qkp.tile([96, QB], BF16, tag="qkT")
                rtmp = qkp.tile([96, QB], BF16, tag="rtmp")
                nc.vector.tensor_mul(qkT[:], p_qk[:], cos_sb[:, sl])
                nc.vector.tensor_mul(rtmp[:], p_qkr[:], sin_sb[:, sl])
                nc.vector.tensor_add(qkT[:], qkT[:], rtmp[:])

                # replicate qT (per head) and kT across the 4 row bands
                qt_h = [qtp.tile([128, QB], BF16, tag=f"qt{h}", name=f"qt{h}")
                        for h in range(2)]
                for t in range(4):
                    dst = slice(32 * t, 32 * (t + 1))
                    nc.sync.dma_start(out=qt_h[0][dst, :], in_=qkT[0:32, :])
                    nc.sync.dma_start(out=qt_h[1][dst, :], in_=qkT[32:64, :])
                    nc.sync.dma_start(out=kT_rep[dst, sl], in_=qkT[64:96, :])

                # v projection for this block's 4 s-chunks
                for s4 in range(4):
                    sblk = 4 * qb + s4
                    pv_ps = ps_mm.tile([128, HD], F32, tag="mm", name="vproj")
                    for c in range(2):
                        nc.tensor.matmul(
                            pv_ps[:], hsT_sb[:, c, KC * sblk:KC * (sblk + 1)],
                            wvT_sb[:, c, :], start=(c == 0), stop=(c == 1))
                    nc.vector.tensor_copy(v_all[:, sblk, 1:], pv_ps[:])
                qt_state[qb] = qt_h

            emit_block_prologue(0)

            for qb in range(NQB):
                q0 = QB * qb
                sl = slice(q0, q0 + QB)
                qt_h = qt_state.pop(qb)

                # ---- previous block: reciprocals (slow DVE) issued at block
                # head; consumed by emit_norm at step dp
                if qb > 0:
                    st = pipe[qb - 1]
                    for h in range(2):
                        rc = smallp.tile([1, QB], BF16, tag="rc", name="rc")
                        with nc.allow_low_precision(reason="bf16 softmax denom"):
                            nc.vector.reciprocal(out=rc[:], in_=st["dn"][h][:])
                        st["rc"].append(rc)

                # ---- attention per head
                ngroups = 2 * (qb + 1)
                # defer the previous block's norm far enough into this block's
                # attention that the reciprocals have finished on DVE
                dp = min(ngroups - 1, 9) if qb > 0 else -1
                pvaccs = [ps_pv.tile([HD + 1, QB], F32, tag="pvacc",
                                     name=f"pvacc{h}") for h in range(2)]

                def emit_pv(st_):
                    g_, h_, pt_ = st_
                    for d in range(2):
                        ch = 2 * g_ + d
                        dg = ch - 4 * qb
                        diag_ = (g_ >= 2 * qb)
                        cols = KC * dg if diag_ else 0
                        nc.tensor.matmul(
                            pvaccs[h_][:, cols:],
                            v_all[:, ch, :],
                            pt_[:, QB * d + cols:QB * (d + 1)],
                            start=(g_ == 0 and d == 0),
                            stop=(g_ == 2 * qb + 1 and d == 1),
                            skip_group_check=True)
                    if g_ < 2 * qb + 1:
                        # zero-adding filler: pads warm-clock PE work up to the
                        # ACT pace so the engine never idles and the HAM clock
                        # gate stays at 2.4GHz (cold PE runs 2x slower).
                        nc.tensor.matmul(
                            pvaccs[h_][:], wtmp[:, 0:HD + 1], wtmp[:],
                            start=False, stop=False, skip_group_check=True)

                # PV pairs run two steps behind their scores so the PE never
                # waits on the exp: while act(i) runs, the PE does pv(i-2),
                # sc(i+1), pv(i-1) -- it reaches pv(i) well after act(i) ends.
                defer = []
                for g in range(ngroups):
                    for h in range(2):
                        sc = ps_sc.tile([128, 2 * QB], F32, tag="sc", name="sc")
                        # causal: chunk dg of the diagonal group only needs
                        # q >= KC*dg; trim matmul + act to that region
                        qlo = [0, 0]
                        if g == 2 * qb:
                            qlo = [0, KC]
                        elif g == 2 * qb + 1:
                            qlo = [2 * KC, 3 * KC]
                        for d in range(2):
                            ch = 2 * g + d
                            t = ch % 4
                            nc.tensor.matmul(
                                sc[:, QB * d + qlo[d]:QB * (d + 1)],
                                kT_rep[32 * t:32 * (t + 1), KC * ch:KC * (ch + 1)],
                                qt_h[h][32 * t:32 * (t + 1), qlo[d]:],
                                start=True, stop=True, tile_position=(32 * t, 0))
                        pt = ptp.tile([128, 2 * QB], BF16, tag="pt", name="pt")
                        nc.scalar.activation(out=pt[:, qlo[0]:], in_=sc[:, qlo[0]:],
                                             func=mybir.ActivationFunctionType.Exp,
                                             scale=float(SCALE))
                        if g >= 2 * qb:
                            for d in range(2):
                                dg = 2 * g + d - 4 * qb
                                w0 = QB * d + KC * dg
                                nc.vector.tensor_mul(
                                    pt[:, w0:w0 + KC], pt[:, w0:w0 + KC], tri_sb[:])
                        defer.append((g, h, pt))
                        if len(defer) > 2:
                            emit_pv(defer.pop(0))
                    if g == dp:
                        emit_norm(qb - 1)
                    if g == ngroups - 2 and qb + 1 < NQB:
                        emit_block_prologue(qb + 1)
                for st_ in defer:
                    emit_pv(st_)

                # ---- epilogue: cheap DVE reads free the PSUM accumulators
                # fast; the reciprocal itself is deferred into the next block
                st = {"u": [], "dn": [], "rc": []}
                for h in range(2):
                    u = up.tile([HD + 1, QB], BF16, tag=f"u{h}", name=f"u{h}")
                    with nc.allow_low_precision(reason="unnorm attn bf16"):
                        nc.vector.tensor_copy(u[:], pvaccs[h][:])
                    dn = smallp.tile([1, QB], F32, tag="dn", name="dn")
                    nc.vector.tensor_copy(dn[:], pvaccs[h][0:1, :])
                    st["u"].append(u)
                    st["dn"].append(dn)
                pipe[qb] = st

            # ---- final block: reciprocal + norm tail
            st = pipe[NQB - 1]
            for h in range(2):
                rc = smallp.tile([1, QB], BF16, tag="rc", name="rc")
                with nc.allow_low_precision(reason="bf16 softmax denom"):
                    nc.vector.reciprocal(out=rc[:], in_=st["dn"][h][:])
                st["rc"].append(rc)
            emit_norm(NQB - 1)

    _install_waitfix(nc)
    return nc


_NC_CACHE = {}


def _get_nc():
    if "nc" not in _NC_CACHE:
        _NC_CACHE["nc"] = _build_module()
    return _NC_CACHE["nc"]


# ---------------------------------------------------------------- host side
def _rope_tables():
    inv = 1.0 / (ROPE_BASE ** (np.arange(0, HD, 2, dtype=np.float64) / HD))
    t = np.arange(S, dtype=np.float64)
    freqs = np.outer(t, inv)                     # [S, 16]
    emb = np.concatenate([freqs, freqs], axis=1)  # [S, 32]
    cosT = np.cos(emb).T.astype(np.float32)      # [32, S]
    sinT = np.sin(emb).T.astype(np.float32)
    return np.tile(cosT, (3, 1)), np.tile(sinT, (3, 1))   # [96, S]


def _rot_rows(w):
    # rows of (rotate_half o) projection: row d<16 -> -w[d+16]; d>=16 -> w[d-16]
    return np.concatenate([-w[16:32], w[0:16]], axis=0)


def kernel(hidden_states, Wq, Wk, Wv, Wo):
    hidden_states = np.asarray(hidden_states, dtype=np.float32)
    Wq = np.asarray(Wq, dtype=np.float32)
    Wk = np.asarray(Wk, dtype=np.float32)
    Wv = np.asarray(Wv, dtype=np.float32)
    Wo = np.asarray(Wo, dtype=np.float32)

    cosT, sinT = _rope_tables()
    tri = (np.arange(KC)[:, None] <= np.arange(KC)[None, :]).astype(NP_BF16)

    hsT_b = [np.ascontiguousarray(hidden_states[b].T).astype(NP_BF16)
             for b in range(B)]

    in_maps = []
    for core in range(8):
        b, j = core // 4, core % 4
        Wq_h = Wq[64 * j:64 * j + 64]            # [64, 256]
        Wk_j = Wk[32 * j:32 * j + 32]            # [32, 256]
        Wqk = np.concatenate([Wq_h, Wk_j], axis=0)           # [96, 256]
        Wqkrot = np.concatenate(
            [_rot_rows(Wq_h[0:32]), _rot_rows(Wq_h[32:64]), _rot_rows(Wk_j)],
            axis=0)
        G = Wo[:, 64 * j:64 * j + 64]            # [256, 64]
        gt0 = np.zeros((HD + 1, H), np.float32)
        gt0[1:] = G[:, 0:32].T
        gt1 = np.zeros((HD + 1, H), np.float32)
        gt1[1:] = G[:, 32:64].T
        in_maps.append({
            "hsT": hsT_b[b],
            "wqkT": np.ascontiguousarray(Wqk.T).astype(NP_BF16),
            "wqkrotT": np.ascontiguousarray(Wqkrot.T).astype(NP_BF16),
            "wvT": np.ascontiguousarray(Wv[32 * j:32 * j + 32].T).astype(NP_BF16),
            "gt0": gt0.astype(NP_BF16),
            "gt1": gt1.astype(NP_BF16),
            "cosT": cosT.astype(NP_BF16),
            "sinT": sinT.astype(NP_BF16),
            "tri": tri,
        })

    nc = _get_nc()
    res = run_bass_kernel_spmd(nc, in_maps, list(range(8)), trace=False)

    out = np.empty((B, S, H), np.float32)
    for b in range(B):
        acc = np.zeros((H, S), np.float32)
        for j in range(4):
            acc += np.asarray(res.results[4 * b + j]["out_part"],
                              dtype=np.float32)
        out[b] = acc.T
    return out
